# revision 1
# baseline (speedup 1.0000x reference)
"""PaDiM AnomalyMapGenerator kernel for 8 TRN2 NeuronCores.

Pipeline (per the reference):
  1. delta = embedding - mean                                   [B, C, HW]
  2. q[n, b] = delta_nb^T @ inv_cov_n @ delta_nb  (per pixel)   [HW, B]
  3. dist = sqrt(max(q, 0)) -> score maps                       [B, 56, 56]
  4. bilinear 4x upsample + 33x33 gaussian blur (reflect)       [B, 1, 224, 224]

Stage 4 is a fixed linear operator, so it collapses to out_b = M @ S_b @ M^T
with M = Blur(224x224, reflect) @ Resize(224x56) precomputed on host.

Sharding: pixels (HW) are split across the 8 cores for stages 1-3 (the 500MB
inv_covariance read dominates; its symmetry is exploited to skip the redundant
lower-left block).  A tiny AllGather (50KB/core) replicates the score maps,
then stage 4 is sharded by OUTPUT ROWS: core j computes rows [28j, 28j+28) of
every image, with its M^T column slice passed as per-core input data so the
SPMD graph is identical across cores.  Compute runs in bf16 on the
TensorEngine (inputs cast on-chip); measured ~372us on silicon, rel err 1.6e-3
vs the f32 reference (gate 2e-2).
"""

import os
import sys

import numpy as np

for _p in ("/opt/trn_rl_repo", "/root/.axon_site/_ro/trn_rl_repo"):
    if os.path.isdir(_p) and _p not in sys.path:
        sys.path.append(_p)

import concourse.bacc as bacc
import concourse.mybir as mybir
import concourse.tile as tile
from concourse.bass_utils import run_bass_kernel_spmd
from concourse.masks import make_identity


def make_scaled_identity(nc, identity, fill):
    nc.gpsimd.memset(identity, 0.0)
    nc.gpsimd.affine_select(
        out=identity, in_=identity,
        compare_op=mybir.AluOpType.not_equal,
        fill=fill, base=0,
        pattern=[[-1, identity.shape[0]]],
        channel_multiplier=1)

F32 = mybir.dt.float32
BF16 = mybir.dt.bfloat16

B, C, H, W = 32, 200, 56, 56
HW = H * W                 # 3136
NCORES = 8
NL = HW // NCORES          # 392 pixels per core
KA, KB = 128, 72           # contraction (channel) split
IMG = 224
SIGMA = 4.0
KS = 33
PAD = (KS - 1) // 2

GRP = 4                    # pixels per matmul group (PE column tiling)
NGRP = NL // GRP           # 98 groups; group g covers pixels {p*98+g}
IPC = B // NCORES          # images per core (legacy)
RPC = IMG // NCORES        # output rows per core (post-proc row sharding)


def _mt_matrix():
    """M^T [56, 224] for out_b = M @ S_b @ M^T == blur(upsample(S_b))."""
    # Bilinear resize weights, jax.image.resize convention (half-pixel centers,
    # triangle kernel, per-row renormalization).  Upsampling -> no antialiasing.
    scale = IMG / H
    u = (np.arange(IMG, dtype=np.float64) + 0.5) / scale - 0.5
    k = np.arange(H, dtype=np.float64)
    wts = np.maximum(0.0, 1.0 - np.abs(u[:, None] - k[None, :]))
    R = wts / wts.sum(axis=1, keepdims=True)          # [224, 56]
    # Gaussian blur with reflect padding as a dense matrix.
    x = np.arange(KS, dtype=np.float64) - (KS - 1) / 2.0
    g = np.exp(-(x * x) / (2.0 * SIGMA * SIGMA))
    g = g / g.sum()
    Bm = np.zeros((IMG, IMG), dtype=np.float64)
    for i in range(IMG):
        for j in range(KS):
            t = i + j - PAD
            if t < 0:
                t = -t
            if t >= IMG:
                t = 2 * IMG - 2 - t
            Bm[i, t] += g[j]
    M = Bm @ R                                         # [224, 56]
    return np.ascontiguousarray(M.T, dtype=np.float32)  # [56, 224]


def build():
    nc = bacc.Bacc("TRN2", target_bir_lowering=False, debug=False,
                   num_devices=NCORES)
    emb = nc.dram_tensor("embedding", [B, C, NL], F32, kind="ExternalInput").ap()
    mean = nc.dram_tensor("mean", [C, NL], F32, kind="ExternalInput").ap()
    icov = nc.dram_tensor("inv_covariance", [NL, C, C], F32,
                          kind="ExternalInput").ap()
    mt = nc.dram_tensor("mt", [H, IMG], F32, kind="ExternalInput").ap()
    # per-core slice of M^T: columns [28j, 28j+28) — the output-row shard
    mtj = nc.dram_tensor("mtj", [H, RPC], F32, kind="ExternalInput").ap()
    outp = nc.dram_tensor("out", [B, RPC, IMG], F32, kind="ExternalOutput").ap()

    # stride-98 pixel grouping: pixel n = p*98 + g, group g handles the 4
    # pixels {g, 98+g, 196+g, 294+g} in PE column-groups p=0..3.  This makes
    # the PSUM (p, b)-partition layout land in dist as contiguous 98-column
    # runs (no 4-byte scatter on the relayout).
    emb_q = emb.rearrange("b i (p g) -> i b p g", p=GRP)     # [C, B, 4, 98]
    icov_q = icov.rearrange("(p g) i j -> i p g j", p=GRP)   # [C, 4, 98, C]
    mean_q = mean.rearrange("i (p g) -> i p g", p=GRP)       # [C, 4, 98]

    with tile.TileContext(nc) as tc:
        with (
            tc.tile_pool(name="const", bufs=1) as cpool,
            tc.tile_pool(name="dtf", bufs=2) as dtfpool,
            tc.tile_pool(name="dt", bufs=3) as dtpool,
            tc.tile_pool(name="ic", bufs=5) as icpool,
            tc.tile_pool(name="icbf", bufs=5) as icbfpool,
            tc.tile_pool(name="scr", bufs=2) as scrpool,
            tc.tile_pool(name="post", bufs=3) as postpool,
            tc.tile_pool(name="psmd", bufs=2, space="PSUM") as psmd,
            tc.tile_pool(name="psd", bufs=2, space="PSUM") as psd,
            tc.tile_pool(name="psw", bufs=2, space="PSUM") as pswpool,
            tc.tile_pool(name="pso", bufs=2, space="PSUM") as psopool,
            tc.tile_pool(name="dram", bufs=1, space="DRAM") as dram,
        ):
            # ---- one-time loads --------------------------------------------
            Ea = cpool.tile([KA, B, GRP, NGRP], F32, tag="Ea")
            Eb = cpool.tile([KB, B, GRP, NGRP], F32, tag="Eb")
            ma = cpool.tile([KA, GRP, NGRP], F32, tag="ma")
            mb = cpool.tile([KB, GRP, NGRP], F32, tag="mb")
            mts = cpool.tile([H, IMG], F32, tag="mts")
            mtsbf = cpool.tile([H, IMG], BF16, tag="mtsbf")
            mtjs = cpool.tile([H, RPC], F32, tag="mtjs")
            mtjbf = cpool.tile([H, RPC], BF16, tag="mtjbf")
            id128 = cpool.tile([KA, KA], BF16, tag="id128")
            id72 = cpool.tile([KB, KB], BF16, tag="id72")
            Q = cpool.tile([128, NGRP], F32, tag="Q")
            Qs = cpool.tile([128, NGRP], F32, tag="Qs")
            dist_sb = cpool.tile([B, NL], F32, tag="dist")

            # big embedding loads go on the ACT HWDGE ring so the sync ring
            # can start streaming inv_cov immediately; flat (b, n) views keep
            # the DMA descriptors at 1568B
            emb_t = emb.rearrange("b i n -> i b n")
            nc.scalar.dma_start(ma[:].rearrange("p x y -> p (x y)"),
                                mean[0:KA])
            nc.scalar.dma_start(mb[:].rearrange("p x y -> p (x y)"),
                                mean[KA:C])
            nc.scalar.dma_start(mts[:], mt)
            nc.scalar.dma_start(mtjs[:], mtj)
            nc.scalar.dma_start(Ea[:].rearrange("p b x y -> p b (x y)"),
                                emb_t[0:KA])
            nc.scalar.dma_start(Eb[:].rearrange("p b x y -> p b (x y)"),
                                emb_t[KA:C])
            nc.vector.tensor_copy(mtsbf[:], mts[:])
            nc.vector.tensor_copy(mtjbf[:], mtjs[:])
            make_identity(nc, id128[:])
            # delta_b is stored pre-scaled by 0.5 (for the symmetric-D trick);
            # 4*I here makes the PSUM delta copy come out as 2*delta_b, which
            # doubles the cross term delta_a^T B delta_b exactly as symmetry
            # requires while keeping delta_b^T D delta_b at 1x.
            make_scaled_identity(nc, id72[:], 4.0)

            # ---- per-pixel mahalanobis -------------------------------------
            DCH = 14                   # groups per delta chunk (batched DVE)
            dTa = dTb = None
            for g in range(NGRP):
                if g % DCH == 0:
                    # delta for 14 groups at once: f32 subtract (fast DVE
                    # path), then flat 2D bf16 cast.  Batching amortizes the
                    # per-op DVE overhead.
                    c0 = g
                    dTaf = dtfpool.tile([KA, B, GRP, DCH], F32, tag="dtaf")
                    dTbf = dtfpool.tile([KB, B, GRP, DCH], F32, tag="dtbf")
                    nc.vector.tensor_sub(
                        dTaf[:], Ea[:, :, :, c0:c0 + DCH],
                        ma[:, :, c0:c0 + DCH].unsqueeze(1).broadcast_to(
                            (KA, B, GRP, DCH)))
                    nc.vector.tensor_sub(
                        dTbf[:], Eb[:, :, :, c0:c0 + DCH],
                        mb[:, :, c0:c0 + DCH].unsqueeze(1).broadcast_to(
                            (KB, B, GRP, DCH)))
                    dTa = dtpool.tile([KA, B, GRP, DCH], BF16, tag="dta")
                    dTb = dtpool.tile([KB, B, GRP, DCH], BF16, tag="dtb")
                    nc.vector.tensor_copy(
                        dTa[:].rearrange("p b x y -> p (b x y)"),
                        dTaf[:].rearrange("p b x y -> p (b x y)"))
                    # delta_b scaled by 0.5 (exact in bf16) — see id72 comment
                    nc.vector.tensor_scalar_mul(
                        dTb[:].rearrange("p b x y -> p (b x y)"),
                        dTbf[:].rearrange("p b x y -> p (b x y)"), 0.5)

                # IC is symmetric: read rows 0:128 fully, but rows 128:200
                # only need their diagonal 72x72 block D (the lower-left
                # 72x128 block duplicates B^T and is folded via the 2x scale)
                ica = icpool.tile([KA, GRP, C], F32, tag="ica")
                icb = icpool.tile([KB, GRP, KB], F32, tag="icb")
                nc.sync.dma_start(ica[:], icov_q[0:KA, :, g, :])
                nc.sync.dma_start(icb[:], icov_q[KA:C, :, g, KA:C])
                # inv_cov cast f32 -> bf16: DVE (fast) for the 128-chunk,
                # ACT for the 72-chunk; flat 2D APs keep both on fast paths
                icabf = icbfpool.tile([KA, GRP, C], BF16, tag="icabf")
                icbbf = icbfpool.tile([KB, GRP, KB], BF16, tag="icbbf")
                # alternate the big cast between DVE and ACT to balance load
                eng_a = nc.vector.tensor_copy if g % 2 == 0 else nc.scalar.copy
                eng_b = nc.scalar.copy if g % 2 == 0 else nc.vector.tensor_copy
                eng_a(icabf[:].rearrange("p x y -> p (x y)"),
                      ica[:].rearrange("p x y -> p (x y)"))
                eng_b(icbbf[:].rearrange("p x y -> p (x y)"),
                      icb[:].rearrange("p x y -> p (x y)"))

                # PSUM tiles use the full 2KB bank row (512 f32) so each
                # pixel's 32-partition slice is a distinct HW zero region.
                ps_md = psmd.tile([128, 512], F32, tag="psmd")
                ps_d = psd.tile([128, 512], F32, tag="psd")
                for p in range(GRP):
                    wa = dTa[:, :, p, g - c0]   # [KA, B] strided
                    wb = dTb[:, :, p, g - c0]   # [KB, B]
                    tp = (0, 32 * p)
                    # md cols 0:200 = delta_a^T [A | B]; cols 128:200 +=
                    # (delta_b/2)^T D  (symmetric-D trick)
                    nc.tensor.matmul(ps_md[32 * p:32 * p + 32, 0:C], wa,
                                     icabf[:, p, :],
                                     start=True, stop=False, tile_position=tp)
                    nc.tensor.matmul(ps_md[32 * p:32 * p + 32, KA:C], wb,
                                     icbbf[:, p, :],
                                     start=False, stop=True, tile_position=tp)
                    # delta copied to PSUM in [b, i] layout via identity mm
                    # (one accumulation group, disjoint column ranges)
                    nc.tensor.matmul(ps_d[32 * p:32 * p + 32, 0:KA], wa,
                                     id128[:], start=True, stop=False,
                                     tile_position=tp)
                    nc.tensor.matmul(ps_d[32 * p:32 * p + 32, KA:C], wb,
                                     id72[:], start=False, stop=True,
                                     tile_position=tp)
                d_sb = scrpool.tile([128, C], F32, tag="dsb")
                nc.scalar.copy(d_sb[:], ps_d[:, 0:C])
                scr = scrpool.tile([128, C], F32, tag="scr")
                # q = sum_j md * delta (InstTensorTensorReduce crashes this
                # runtime, so multiply + reduce as two DVE ops)
                nc.vector.tensor_mul(scr[:], ps_md[:, 0:C], d_sb[:])
                nc.vector.reduce_sum(Q[:, g:g + 1], scr[:],
                                     axis=mybir.AxisListType.X)

            # ---- dist = sqrt(relu(q)); relayout [(p,b), g] -> [b, p*98+g] --
            nc.vector.tensor_scalar_max(Q[:], Q[:], 0.0)
            nc.scalar.sqrt(Qs[:], Q[:])
            dview = dist_sb[:].rearrange("b (p g) -> p b g", p=GRP)
            for p in range(GRP):
                nc.sync.dma_start(dview[p], Qs[32 * p:32 * p + 32, :])

            # ---- AllGather score maps; post-proc sharded by output rows ----
            dist_dram = dram.tile([B, NL], F32, tag="dist_dram")
            gall = dram.tile([NCORES * B, NL], F32, tag="gall")
            s_dram = dram.tile([B, H, W], F32, tag="s_dram")
            nc.sync.dma_start(dist_dram[:], dist_sb[:])
            nc.gpsimd.collective_compute(
                "AllGather", mybir.AluOpType.bypass,
                replica_groups=[list(range(NCORES))],
                ins=[dist_dram[:].opt()],
                outs=[gall[:].opt()],
            )
            # s_dram[b, 7*sc+rl, c] = gall[32*sc + b, 56*rl + c]
            nc.sync.dma_start(
                s_dram[:].rearrange("b (sc rl) c -> b sc rl c", sc=NCORES),
                gall[:].rearrange("(sc b) (rl c) -> b sc rl c", b=B, c=W))

            # out rows [28j:28j+28) of M @ S_b @ M^T for ALL images:
            #   W = S_b^T @ mtj   ([56, 28], mtj = per-core M^T column slice)
            #   out = W^T @ M^T   ([28, 224])
            sk_f = cpool.tile([H, B, W], F32, tag="sk_f")
            sk_all = cpool.tile([H, B, W], BF16, tag="sk_all")
            nc.sync.dma_start(sk_f[:], s_dram[:].rearrange("b r c -> r b c"))
            nc.vector.tensor_copy(sk_all[:].rearrange("p b c -> p (b c)"),
                                  sk_f[:].rearrange("p b c -> p (b c)"))
            # 4 images per PSUM bank: W_t at column quarters, then one mm2
            # computes the 28 output rows of all 4 images at once
            for t0 in range(0, B, 4):
                psw = pswpool.tile([H, 512], F32, tag="psw")
                for t in range(4):
                    nc.tensor.matmul(psw[:, 128 * t:128 * t + RPC],
                                     sk_all[:, t0 + t, :], mtjbf[:],
                                     start=(t == 0), stop=(t == 3))
                wsb = postpool.tile([H, 4, RPC], BF16, tag="wsb")
                nc.scalar.copy(
                    wsb[:],
                    psw[:].rearrange("p (x y) -> p x y", x=4)[:, :, 0:RPC])
                pso = psopool.tile([4 * RPC, 512], F32, tag="pso")
                nc.tensor.matmul(pso[:, 0:IMG],
                                 wsb[:].rearrange("p x y -> p (x y)"),
                                 mtsbf[:], start=True, stop=True)
                osb = postpool.tile([4 * RPC, IMG], F32, tag="osb")
                nc.vector.tensor_copy(osb[:], pso[:, 0:IMG])
                nc.sync.dma_start(
                    outp[t0:t0 + 4].rearrange("t i j -> (t i) j"), osb[:])

    nc.compile()
    return nc


_NC = None


def _get_nc():
    global _NC
    if _NC is None:
        _NC = build()
    return _NC


def make_in_maps(embedding, mean, inv_covariance):
    emb = np.ascontiguousarray(
        np.asarray(embedding, dtype=np.float32).reshape(B, C, HW))
    mean = np.asarray(mean, dtype=np.float32)
    icov = np.asarray(inv_covariance, dtype=np.float32)
    mt = _mt_matrix()
    in_maps = []
    for i in range(NCORES):
        sl = slice(i * NL, (i + 1) * NL)
        in_maps.append({
            "embedding": np.ascontiguousarray(emb[:, :, sl]),
            "mean": np.ascontiguousarray(mean[:, sl]),
            "inv_covariance": np.ascontiguousarray(icov[sl]),
            "mt": mt,
            "mtj": np.ascontiguousarray(mt[:, i * RPC:(i + 1) * RPC]),
        })
    return in_maps


def run(embedding, mean, inv_covariance, trace=False):
    nc = _get_nc()
    in_maps = make_in_maps(embedding, mean, inv_covariance)
    res = run_bass_kernel_spmd(nc, in_maps, list(range(NCORES)), trace=trace)
    # core i returns out rows [28i, 28i+28) for all images
    full = np.concatenate([res.results[i]["out"] for i in range(NCORES)],
                          axis=1).reshape(B, 1, IMG, IMG)
    return np.ascontiguousarray(full, dtype=np.float32), res


def kernel(embedding, mean, inv_covariance, covariance=None):
    out, _ = run(embedding, mean, inv_covariance, trace=False)
    return out

